# revision 4
# baseline (speedup 1.0000x reference)
"""Trainium2 Bass kernel for nn_DefuzzyLayer: out = x @ rules_outcome.

x: [8192, 4096] f32, rules_outcome: [4096, 4096] f32 -> out: [8192, 4096] f32.

Strategy: data-parallel over batch. Each of the 8 NeuronCores computes a
[1024, 4096] output shard (stored transposed as [4096, 1024]) with the full
W replicated.

Per-core kernel (Tile framework), v2 — W-stationary:
  - Inputs cast to fp16 host-side (PSUM accumulation stays fp32; input
    rounding gives ~3e-4 relative error on the output).
  - Stationary operand = W k-tile x n-tile [128, 128]; moving operand =
    x^T [128k, 1024m] (two 512-column matmuls per stationary load, PSUM
    bank limit). One stationary load serves 1024 moving columns, so
    LDWEIGHTS is trivially hidden and there are only 2048 matmuls of
    N=512 per core.
  - psum [128n, 512m] accumulates over all 32 k-tiles, then is evicted
    per n-tile (DVE) and stored per n-tile (scalar ring) -> low tail.
  - DMA: W n-tile blocks (1 MiB each) stream on the sync ring,
    triple-buffered; x^T chunks load once on gpsimd+vector rings in
    parallel so the first matmul starts after ~2 MiB of DMA.
  - Output is out^T [4096, 1024] f32 per core; host unshard transposes.
"""

import contextlib
import os

import numpy as np

BATCH = 8192
IN_DIM = 4096
OUT_DIM = 4096
N_CORES = 8
M_SHARD = BATCH // N_CORES  # 1024

P = 128
KT = IN_DIM // P       # 32 k-tiles
NT = OUT_DIM // P      # 32 n-tiles
MH = 2                 # m halves (512 each; PSUM bank = 512 fp32)
MHS = M_SHARD // MH    # 512

XCHUNKS = int(os.environ.get("KXC", "8"))
KPC = KT // XCHUNKS    # k-tiles per x chunk
WBUFS = int(os.environ.get("KWB", "4"))
PSBUFS = int(os.environ.get("KPSBUFS", "8"))
OBUFS = int(os.environ.get("KOB", "3"))

IN_DT = os.environ.get("KDT", "float16")  # float16 | bfloat16 | float32r

_cached_nc = None


def _np_dt():
    if IN_DT == "float16":
        return np.float16
    if IN_DT == "bfloat16":
        import ml_dtypes
        return np.dtype(ml_dtypes.bfloat16)
    return np.float32


def _build(loop_n=1, in_dt=None):
    """Build + compile the per-core Bass module.

    loop_n > 1 wraps the whole body in an on-device For_i loop — used only
    for HW timing (amortizes host dispatch overhead out of the measurement).
    """
    import concourse.bacc as bacc
    import concourse.tile as tile
    import concourse.mybir as mybir

    dt_in = getattr(mybir.dt, in_dt or IN_DT)

    nc = bacc.Bacc("TRN2", target_bir_lowering=False, debug=False)
    # packed inputs (see _pack_x_shard/_pack_w)
    xt = nc.dram_tensor(
        "xt", [P, KT * M_SHARD], dt_in, kind="ExternalInput"
    ).ap()
    w = nc.dram_tensor(
        "w", [P, NT * KT * P], dt_in, kind="ExternalInput"
    ).ap()
    out = nc.dram_tensor(
        "out", [OUT_DIM, M_SHARD], mybir.dt.float32, kind="ExternalOutput"
    ).ap()
    out_r = out.rearrange("(n p) m -> p n m", p=P)  # [128, NT, M_SHARD]

    with tile.TileContext(nc) as tc:
        loop_ctx = (
            tc.For_i(0, loop_n, 1,
                     hint_engines=(mybir.EngineType.PE, mybir.EngineType.SP,
                                   mybir.EngineType.DVE))
            if loop_n > 1 else contextlib.nullcontext()
        )
        with (
            loop_ctx,
            tc.tile_pool(name="xpool", bufs=XCHUNKS) as xpool,
            tc.tile_pool(name="wpool", bufs=WBUFS) as wpool,
            tc.tile_pool(name="opool", bufs=OBUFS) as opool,
            tc.tile_pool(name="pspool", bufs=PSBUFS, space="PSUM") as pspool,
        ):
            # Startup: interleave w / x loads on the sync ring so the first
            # matmul starts after ~2 MiB and the first GA n-tiles can
            # proceed k-progressively as x chunks land.
            w_tiles = {}
            x_chunks = []

            def load_w(n):
                w_n = wpool.tile([P, KT * P], dt_in, name=f"w{n}", tag="w")
                nc.sync.dma_start(
                    out=w_n[:],
                    in_=w[:, n * KT * P:(n + 1) * KT * P],
                )
                w_tiles[n] = w_n
                return w_n

            def load_x(c):
                x_c = xpool.tile([P, KPC * M_SHARD], dt_in,
                                 name=f"x{c}", tag="x")
                nc.sync.dma_start(
                    out=x_c[:],
                    in_=xt[:, c * KPC * M_SHARD:(c + 1) * KPC * M_SHARD],
                )
                x_chunks.append(x_c)

            psums = {}

            def get_psums(n):
                psums[n] = [
                    pspool.tile([P, MHS], mybir.dt.float32,
                                name=f"ps{n}_{h}", tag="ps")
                    for h in range(MH)
                ]

            def mm(n, k):
                xc = x_chunks[k // KPC]
                koff = (k % KPC) * M_SHARD
                for h in range(MH):
                    nc.tensor.matmul(
                        psums[n][h][:],
                        w_tiles[n][:, k * P:(k + 1) * P],
                        xc[:, koff + h * MHS:koff + (h + 1) * MHS],
                        start=(k == 0),
                        stop=(k == KT - 1),
                    )

            def evict(n):
                o_n = opool.tile([P, M_SHARD], mybir.dt.float32,
                                 name=f"o{n}", tag="o")
                for h in range(MH):
                    nc.vector.tensor_copy(
                        o_n[:, h * MHS:(h + 1) * MHS], psums[n][h][:])
                nc.scalar.dma_start(out=out_r[:, n, :], in_=o_n[:])
                del psums[n]

            # interleaved issue order: w0, x0, w1, x1, w2, x2, w3, x3,
            # x4..x7, then w4.. as the n-loop proceeds (wpool prefetch).
            GA = min(4, NT)
            for i in range(max(GA, XCHUNKS)):
                if i < GA:
                    load_w(i)
                if i < XCHUNKS:
                    load_x(i)

            # Phase A: first GA n-tiles advance k-progressively per x chunk
            for n in range(GA):
                get_psums(n)
            for c in range(XCHUNKS):
                for n in range(GA):
                    for k in range(c * KPC, (c + 1) * KPC):
                        mm(n, k)
            for n in range(GA):
                evict(n)

            # Phase B: remaining n-tiles, serial (2 PSUM banks each)
            for n in range(GA, NT):
                load_w(n)
                get_psums(n)
                for k in range(KT):
                    mm(n, k)
                evict(n)

    nc.compile()
    return nc


def _get_nc():
    global _cached_nc
    if _cached_nc is None:
        _cached_nc = _build()
    return _cached_nc


def _pack_x_shard(x_shard):
    """[M_SHARD, IN_DIM] -> [128, KT*M_SHARD] partition-major x^T tiles.

    dest[p, k*M_SHARD + m] = x_shard[m, k*128 + p]
    """
    return np.ascontiguousarray(
        x_shard.T.reshape(KT, P, M_SHARD).transpose(1, 0, 2).reshape(P, -1)
    )


def _pack_w(w_full):
    """[IN_DIM, OUT_DIM] -> [128, NT*KT*128] n-tile-major k-tile blocks.

    dest[p, (n*KT + k)*128 + j] = w_full[k*128 + p, n*128 + j]
    """
    return np.ascontiguousarray(
        w_full.reshape(KT, P, NT, P).transpose(1, 2, 0, 3).reshape(P, -1)
    )


def _make_in_maps(x, rules_outcome):
    np_dt = _np_dt()
    x = np.asarray(x, dtype=np_dt)
    w = np.asarray(rules_outcome, dtype=np_dt)
    assert x.shape == (BATCH, IN_DIM) and w.shape == (IN_DIM, OUT_DIM)
    w_packed = _pack_w(w)
    return [
        {
            "xt": _pack_x_shard(x[i * M_SHARD:(i + 1) * M_SHARD, :]),
            "w": w_packed,
        }
        for i in range(N_CORES)
    ]


def _run(x, rules_outcome, **spmd_kwargs):
    from concourse.bass_utils import run_bass_kernel_spmd

    in_maps = _make_in_maps(x, rules_outcome)
    nc = _get_nc()
    res = run_bass_kernel_spmd(nc, in_maps, core_ids=list(range(N_CORES)),
                               **spmd_kwargs)
    full = np.concatenate(
        [res.results[i]["out"].T for i in range(N_CORES)], axis=0)
    return np.ascontiguousarray(full), res


def kernel(x, rules_outcome):
    out, _ = _run(x, rules_outcome)
    return out


# revision 30
# speedup vs baseline: 1.0635x; 1.0635x over previous
"""Trainium2 Bass kernel for nn_DefuzzyLayer: out = x @ rules_outcome.

x: [8192, 4096] f32, rules_outcome: [4096, 4096] f32 -> out: [8192, 4096] f32.

Strategy: data-parallel over batch. Each of the 8 NeuronCores computes a
[1024, 4096] output shard (stored transposed as [4096, 1024]) with the full
W replicated.

Per-core kernel (Tile framework), v2 — W-stationary:
  - Inputs cast to fp16 host-side (PSUM accumulation stays fp32; input
    rounding gives ~3e-4 relative error on the output).
  - Stationary operand = W k-tile x n-tile [128, 128]; moving operand =
    x^T [128k, 1024m] (two 512-column matmuls per stationary load, PSUM
    bank limit). One stationary load serves 1024 moving columns, so
    LDWEIGHTS hides easily and there are only 2048 matmuls of N=512
    per core (437 us of PE work at 2.4 GHz).
  - All loads share the sync ring in consumer order (w0, x0, w1, x1,
    ..., x7, w4, w5, ...); the first matmul starts after ~2 MiB. The
    first 4 n-tiles advance k-progressively as x chunks land so the PE
    never waits on the bulk x load; afterwards W streams 1 n-tile
    ahead of compute.
  - psum [128n, 512m] accumulates over all 32 k-tiles, then is evicted
    per n-tile (DVE, fp16 cast) and stored per n-tile (scalar ring).
    The last n-tile runs m-half-outer so its eviction+store hide under
    matmuls, shrinking the end-of-iteration tail.
  - Output is out^T [4096, 1024] fp16 per core; host unshard transposes
    and upcasts (total rel err ~3.5e-4 vs fp32 reference).
"""

import contextlib
import os

import numpy as np

BATCH = 8192
IN_DIM = 4096
OUT_DIM = 4096
N_CORES = 8
M_SHARD = BATCH // N_CORES  # 1024

P = 128
KT = IN_DIM // P       # 32 k-tiles
NT = OUT_DIM // P      # 32 n-tiles
MH = 2                 # m halves (512 each; PSUM bank = 512 fp32)
MHS = M_SHARD // MH    # 512

XCHUNKS = int(os.environ.get("KXC", "8"))
KPC = KT // XCHUNKS    # k-tiles per x chunk
WBUFS = int(os.environ.get("KWB", "4"))
PSBUFS = int(os.environ.get("KPSBUFS", "8"))
OBUFS = int(os.environ.get("KOB", "3"))

IN_DT = os.environ.get("KDT", "float16")  # float16 | bfloat16 | float32r
OUT_DT = os.environ.get("KODT", "float16")  # float32 | float16 | bfloat16

_cached_nc = None


def _np_dt():
    if IN_DT == "float16":
        return np.float16
    if IN_DT == "bfloat16":
        import ml_dtypes
        return np.dtype(ml_dtypes.bfloat16)
    return np.float32


def _build(loop_n=1, in_dt=None, variant="full", timing=False):
    """Build + compile the per-core Bass module.

    loop_n > 1 wraps the whole body in an on-device For_i loop — used only
    for HW timing (amortizes host dispatch overhead out of the measurement).
    variant: "full" | "nodma" (memset x/w instead of loading) |
             "mm256" (4 matmuls of 256 cols per k — LDWEIGHTS stress probe)
    timing: make the big output an Internal dram tensor (not transferred
    per call) and expose only a tiny marker output — cuts per-call h2d to
    ~nothing so wall-clock timing is stable.
    """
    import concourse.bacc as bacc
    import concourse.tile as tile
    import concourse.mybir as mybir

    do_in_dma = variant not in ("nodma", "mmonly")
    do_evict = variant != "mmonly"
    do_store = os.environ.get("KSTORE", "1") == "1"
    ev_eng = os.environ.get("KEVENG", "dve")  # dve | act
    ev_half = os.environ.get("KEVHALF", "0") == "1"
    ps_pair = os.environ.get("KPAIR", "0") == "1"  # one [P,1024] psum/n-tile
    n_sub = 2 if variant == "mm256" else 1
    sub = MHS // n_sub

    dt_in = getattr(mybir.dt, in_dt or IN_DT)

    nc = bacc.Bacc("TRN2", target_bir_lowering=False, debug=False)
    # packed inputs (see _pack_x_shard/_pack_w)
    xt = nc.dram_tensor(
        "xt", [P, KT * M_SHARD], dt_in, kind="ExternalInput"
    ).ap()
    w = nc.dram_tensor(
        "w", [P, NT * KT * P], dt_in, kind="ExternalInput"
    ).ap()
    dt_out = getattr(mybir.dt, OUT_DT)
    out = nc.dram_tensor(
        "out", [OUT_DIM, M_SHARD], dt_out,
        kind="Internal" if timing else "ExternalOutput",
    ).ap()
    out_r = out.rearrange("(n p) m -> p n m", p=P)  # [128, NT, M_SHARD]
    marker = None
    if timing:
        marker = nc.dram_tensor(
            "marker", [1, 64], mybir.dt.int32, kind="ExternalOutput"
        ).ap()

    with tile.TileContext(nc) as tc:
        loop_ctx = (
            tc.For_i(0, loop_n, 1,
                     hint_engines=(mybir.EngineType.PE, mybir.EngineType.SP,
                                   mybir.EngineType.DVE))
            if loop_n > 1 else contextlib.nullcontext()
        )
        with (
            loop_ctx,
            tc.tile_pool(name="xpool", bufs=XCHUNKS) as xpool,
            tc.tile_pool(name="wpool", bufs=WBUFS) as wpool,
            tc.tile_pool(name="opool", bufs=OBUFS) as opool,
            tc.tile_pool(name="pspool", bufs=PSBUFS, space="PSUM") as pspool,
        ):
            # Startup: interleave w / x loads on the sync ring so the first
            # matmul starts after ~2 MiB and the first GA n-tiles can
            # proceed k-progressively as x chunks land.
            w_tiles = {}
            x_chunks = []

            def load_w(n):
                w_n = wpool.tile([P, KT * P], dt_in, name=f"w{n}", tag="w")
                if do_in_dma:
                    nc.sync.dma_start(
                        out=w_n[:],
                        in_=w[:, n * KT * P:(n + 1) * KT * P],
                    )
                else:
                    nc.gpsimd.memset(w_n[:, 0:1], 0.0)
                w_tiles[n] = w_n
                return w_n

            def load_x(c):
                x_c = xpool.tile([P, KPC * M_SHARD], dt_in,
                                 name=f"x{c}", tag="x")
                if do_in_dma:
                    nc.sync.dma_start(
                        out=x_c[:],
                        in_=xt[:, c * KPC * M_SHARD:(c + 1) * KPC * M_SHARD],
                    )
                else:
                    nc.gpsimd.memset(x_c[:, 0:1], 0.0)
                x_chunks.append(x_c)

            psums = {}   # n -> list of MM-target APs (one per m-half)
            evsrc = {}   # n -> list of (width, src AP) eviction sources
            shared_ps = None
            if not do_evict:
                shared_ps = [
                    pspool.tile([P, MHS], mybir.dt.float32,
                                name=f"sps{h}", tag="ps")[:]
                    for h in range(MH)
                ]

            def get_psums(n):
                if not do_evict:
                    psums[n] = shared_ps
                    return
                if ps_pair:
                    pp = pspool.tile([P, MH * MHS], mybir.dt.float32,
                                     name=f"ps{n}", tag="ps")
                    psums[n] = [pp[:, h * MHS:(h + 1) * MHS]
                                for h in range(MH)]
                    evsrc[n] = [(MH * MHS, pp[:])]
                else:
                    tiles = [
                        pspool.tile([P, MHS], mybir.dt.float32,
                                    name=f"ps{n}_{h}", tag="ps")
                        for h in range(MH)
                    ]
                    psums[n] = [t[:] for t in tiles]
                    evsrc[n] = [(MHS, t[:]) for t in tiles]

            def mm(n, k):
                xc = x_chunks[k // KPC]
                koff = (k % KPC) * M_SHARD
                for h in range(MH):
                    for s in range(n_sub):
                        nc.tensor.matmul(
                            psums[n][h][:, s * sub:(s + 1) * sub]
                            if n_sub > 1 else psums[n][h],
                            w_tiles[n][:, k * P:(k + 1) * P],
                            xc[:, koff + h * MHS + s * sub:
                               koff + h * MHS + (s + 1) * sub],
                            start=(k == 0),
                            stop=(k == KT - 1),
                        )

            def evict(n):
                if not do_evict:
                    del psums[n]
                    return
                o_n = opool.tile([P, M_SHARD], dt_out, name=f"o{n}", tag="o")
                off = 0
                for i, (width, src) in enumerate(evsrc[n]):
                    if ev_half and i > 0:
                        break
                    dst = o_n[:, off:off + width]
                    if ev_eng == "act":
                        nc.scalar.copy(dst, src)
                    else:
                        nc.vector.tensor_copy(dst, src)
                    off += width
                if do_store:
                    eng = nc.sync if ev_eng == "act" else nc.scalar
                    eng.dma_start(out=out_r[:, n, :], in_=o_n[:])
                del psums[n], evsrc[n]

            # interleaved issue order: w0, x0, w1, x1, w2, x2, w3, x3,
            # x4..x7, then w4.. as the n-loop proceeds (wpool prefetch).
            GA = min(int(os.environ.get("KGA", "4")), NT)
            for i in range(max(GA, XCHUNKS)):
                if i < GA:
                    load_w(i)
                if i < XCHUNKS:
                    load_x(i)

            # Phase A: first GA n-tiles advance k-progressively per x chunk
            for n in range(GA):
                get_psums(n)
            for c in range(XCHUNKS):
                for n in range(GA):
                    for k in range(c * KPC, (c + 1) * KPC):
                        mm(n, k)
            for n in range(GA):
                evict(n)

            # Phase B: remaining n-tiles, serial (2 PSUM banks each).
            # The last n-tile runs m-half-outer so the first half's
            # eviction + store hide under the second half's matmuls,
            # shrinking the end-of-iteration tail.
            last_special = do_evict and n_sub == 1
            for n in range(GA, NT - 1 if last_special else NT):
                load_w(n)
                get_psums(n)
                for k in range(KT):
                    mm(n, k)
                evict(n)

            if last_special:
                n = NT - 1
                load_w(n)
                get_psums(n)
                o_n = opool.tile([P, M_SHARD], dt_out, name=f"o{n}",
                                 tag="o")
                for h in range(MH):
                    for k in range(KT):
                        xc = x_chunks[k // KPC]
                        koff = (k % KPC) * M_SHARD
                        nc.tensor.matmul(
                            psums[n][h],
                            w_tiles[n][:, k * P:(k + 1) * P],
                            xc[:, koff + h * MHS:koff + (h + 1) * MHS],
                            start=(k == 0),
                            stop=(k == KT - 1),
                        )
                    dst = o_n[:, h * MHS:(h + 1) * MHS]
                    if ev_eng == "act":
                        nc.scalar.copy(dst, psums[n][h])
                    else:
                        nc.vector.tensor_copy(dst, psums[n][h])
                    if do_store:
                        eng = nc.sync if ev_eng == "act" else nc.scalar
                        eng.dma_start(
                            out=out_r[:, n, h * MHS:(h + 1) * MHS],
                            in_=dst)
                del psums[n], evsrc[n]

        if timing:
            with tc.tile_pool(name="mpool", bufs=1) as mpool:
                m_t = mpool.tile([1, 64], mybir.dt.int32, name="mk", tag="mk")
                nc.gpsimd.memset(m_t[:], 0)
                nc.sync.dma_start(out=marker[:, :], in_=m_t[:])

    nc.compile()
    return nc


def _get_nc():
    global _cached_nc
    if _cached_nc is None:
        _cached_nc = _build()
    return _cached_nc


def _pack_x_shard(x_shard):
    """[M_SHARD, IN_DIM] -> [128, KT*M_SHARD] partition-major x^T tiles.

    dest[p, k*M_SHARD + m] = x_shard[m, k*128 + p]
    """
    return np.ascontiguousarray(
        x_shard.T.reshape(KT, P, M_SHARD).transpose(1, 0, 2).reshape(P, -1)
    )


def _pack_w(w_full):
    """[IN_DIM, OUT_DIM] -> [128, NT*KT*128] n-tile-major k-tile blocks.

    dest[p, (n*KT + k)*128 + j] = w_full[k*128 + p, n*128 + j]
    """
    return np.ascontiguousarray(
        w_full.reshape(KT, P, NT, P).transpose(1, 2, 0, 3).reshape(P, -1)
    )


def _make_in_maps(x, rules_outcome):
    np_dt = _np_dt()
    x = np.asarray(x, dtype=np_dt)
    w = np.asarray(rules_outcome, dtype=np_dt)
    assert x.shape == (BATCH, IN_DIM) and w.shape == (IN_DIM, OUT_DIM)
    w_packed = _pack_w(w)
    return [
        {
            "xt": _pack_x_shard(x[i * M_SHARD:(i + 1) * M_SHARD, :]),
            "w": w_packed,
        }
        for i in range(N_CORES)
    ]


def _run(x, rules_outcome, **spmd_kwargs):
    from concourse.bass_utils import run_bass_kernel_spmd

    in_maps = _make_in_maps(x, rules_outcome)
    nc = _get_nc()
    res = run_bass_kernel_spmd(nc, in_maps, core_ids=list(range(N_CORES)),
                               **spmd_kwargs)
    full = np.concatenate(
        [res.results[i]["out"].T for i in range(N_CORES)], axis=0)
    return np.ascontiguousarray(full, dtype=np.float32), res


def kernel(x, rules_outcome):
    out, _ = _run(x, rules_outcome)
    return out


# revision 31
# speedup vs baseline: 1.1231x; 1.0561x over previous
"""Trainium2 Bass kernel for nn_DefuzzyLayer: out = x @ rules_outcome.

x: [8192, 4096] f32, rules_outcome: [4096, 4096] f32 -> out: [8192, 4096] f32.

Strategy: data-parallel over batch. Each of the 8 NeuronCores computes a
[1024, 4096] output shard (stored transposed as [4096, 1024]) with the full
W replicated.

Per-core kernel (Tile framework), v2 — W-stationary:
  - Inputs cast to fp16 host-side (PSUM accumulation stays fp32; input
    rounding gives ~3e-4 relative error on the output).
  - Stationary operand = W k-tile x n-tile [128, 128]; moving operand =
    x^T [128k, 1024m] (two 512-column matmuls per stationary load, PSUM
    bank limit). One stationary load serves 1024 moving columns, so
    LDWEIGHTS hides easily and there are only 2048 matmuls of N=512
    per core (437 us of PE work at 2.4 GHz).
  - All loads share the sync ring in consumer order (w0, x0, w1, x1,
    ..., x7, w4, w5, ...); the first matmul starts after ~2 MiB. The
    first 4 n-tiles advance k-progressively as x chunks land so the PE
    never waits on the bulk x load; afterwards W streams 1 n-tile
    ahead of compute.
  - psum [128n, 512m] accumulates over all 32 k-tiles, then is evicted
    per n-tile (DVE, fp16 cast) and stored per n-tile (scalar ring).
    The last n-tile runs m-half-outer so its eviction+store hide under
    matmuls, shrinking the end-of-iteration tail.
  - Output is out^T [4096, 1024] fp16 per core; host unshard transposes
    and upcasts (total rel err ~3.5e-4 vs fp32 reference).
"""

import contextlib
import os

import numpy as np

BATCH = 8192
IN_DIM = 4096
OUT_DIM = 4096
N_CORES = 8
M_SHARD = BATCH // N_CORES  # 1024

P = 128
KT = IN_DIM // P       # 32 k-tiles
NT = OUT_DIM // P      # 32 n-tiles
MH = 2                 # m halves (512 each; PSUM bank = 512 fp32)
MHS = M_SHARD // MH    # 512

XCHUNKS = int(os.environ.get("KXC", "8"))
KPC = KT // XCHUNKS    # k-tiles per x chunk
WBUFS = int(os.environ.get("KWB", "4"))
PSBUFS = int(os.environ.get("KPSBUFS", "8"))
OBUFS = int(os.environ.get("KOB", "3"))

IN_DT = os.environ.get("KDT", "float16")  # float16 | bfloat16 | float32r
OUT_DT = os.environ.get("KODT", "float16")  # float32 | float16 | bfloat16

_cached_nc = None


def _np_dt():
    if IN_DT == "float16":
        return np.float16
    if IN_DT == "bfloat16":
        import ml_dtypes
        return np.dtype(ml_dtypes.bfloat16)
    return np.float32


def _build(loop_n=1, in_dt=None, variant="full", timing=False):
    """Build + compile the per-core Bass module.

    loop_n > 1 wraps the whole body in an on-device For_i loop — used only
    for HW timing (amortizes host dispatch overhead out of the measurement).
    variant: "full" | "nodma" (memset x/w instead of loading) |
             "mm256" (4 matmuls of 256 cols per k — LDWEIGHTS stress probe)
    timing: make the big output an Internal dram tensor (not transferred
    per call) and expose only a tiny marker output — cuts per-call h2d to
    ~nothing so wall-clock timing is stable.
    """
    import concourse.bacc as bacc
    import concourse.tile as tile
    import concourse.mybir as mybir

    do_in_dma = variant not in ("nodma", "mmonly")
    do_evict = variant != "mmonly"
    do_store = os.environ.get("KSTORE", "1") == "1"
    ev_eng = os.environ.get("KEVENG", "dve")  # dve | act
    ev_half = os.environ.get("KEVHALF", "0") == "1"
    ps_pair = os.environ.get("KPAIR", "0") == "1"  # one [P,1024] psum/n-tile
    n_sub = 2 if variant == "mm256" else 1
    sub = MHS // n_sub

    dt_in = getattr(mybir.dt, in_dt or IN_DT)

    nc = bacc.Bacc("TRN2", target_bir_lowering=False, debug=False)
    # packed inputs (see _pack_x_shard/_pack_w)
    xt = nc.dram_tensor(
        "xt", [P, KT * M_SHARD], dt_in, kind="ExternalInput"
    ).ap()
    w = nc.dram_tensor(
        "w", [P, NT * KT * P], dt_in, kind="ExternalInput"
    ).ap()
    dt_out = getattr(mybir.dt, OUT_DT)
    out = nc.dram_tensor(
        "out", [OUT_DIM, M_SHARD], dt_out,
        kind="Internal" if timing else "ExternalOutput",
    ).ap()
    out_r = out.rearrange("(n p) m -> p n m", p=P)  # [128, NT, M_SHARD]
    marker = None
    if timing:
        marker = nc.dram_tensor(
            "marker", [1, 64], mybir.dt.int32, kind="ExternalOutput"
        ).ap()

    with tile.TileContext(nc) as tc:
        loop_ctx = (
            tc.For_i(0, loop_n, 1,
                     hint_engines=(mybir.EngineType.PE, mybir.EngineType.SP,
                                   mybir.EngineType.DVE))
            if loop_n > 1 else contextlib.nullcontext()
        )
        with (
            loop_ctx,
            tc.tile_pool(name="xpool", bufs=XCHUNKS) as xpool,
            tc.tile_pool(name="wpool", bufs=WBUFS) as wpool,
            tc.tile_pool(name="opool", bufs=OBUFS) as opool,
            tc.tile_pool(name="pspool", bufs=PSBUFS, space="PSUM") as pspool,
        ):
            # Startup: interleave w / x loads on the sync ring so the first
            # matmul starts after ~2 MiB and the first GA n-tiles can
            # proceed k-progressively as x chunks land.
            w_tiles = {}
            x_chunks = []

            def load_w(n):
                w_n = wpool.tile([P, KT * P], dt_in, name=f"w{n}", tag="w")
                if do_in_dma:
                    nc.sync.dma_start(
                        out=w_n[:],
                        in_=w[:, n * KT * P:(n + 1) * KT * P],
                    )
                else:
                    nc.gpsimd.memset(w_n[:, 0:1], 0.0)
                w_tiles[n] = w_n
                return w_n

            x_ring = os.environ.get("KXRING", "gpsimd")

            def load_x(c):
                x_c = xpool.tile([P, KPC * M_SHARD], dt_in,
                                 name=f"x{c}", tag="x")
                if do_in_dma:
                    getattr(nc, x_ring).dma_start(
                        out=x_c[:],
                        in_=xt[:, c * KPC * M_SHARD:(c + 1) * KPC * M_SHARD],
                    )
                else:
                    nc.gpsimd.memset(x_c[:, 0:1], 0.0)
                x_chunks.append(x_c)

            psums = {}   # n -> list of MM-target APs (one per m-half)
            evsrc = {}   # n -> list of (width, src AP) eviction sources
            shared_ps = None
            if not do_evict:
                shared_ps = [
                    pspool.tile([P, MHS], mybir.dt.float32,
                                name=f"sps{h}", tag="ps")[:]
                    for h in range(MH)
                ]

            def get_psums(n):
                if not do_evict:
                    psums[n] = shared_ps
                    return
                if ps_pair:
                    pp = pspool.tile([P, MH * MHS], mybir.dt.float32,
                                     name=f"ps{n}", tag="ps")
                    psums[n] = [pp[:, h * MHS:(h + 1) * MHS]
                                for h in range(MH)]
                    evsrc[n] = [(MH * MHS, pp[:])]
                else:
                    tiles = [
                        pspool.tile([P, MHS], mybir.dt.float32,
                                    name=f"ps{n}_{h}", tag="ps")
                        for h in range(MH)
                    ]
                    psums[n] = [t[:] for t in tiles]
                    evsrc[n] = [(MHS, t[:]) for t in tiles]

            def mm(n, k):
                xc = x_chunks[k // KPC]
                koff = (k % KPC) * M_SHARD
                for h in range(MH):
                    for s in range(n_sub):
                        nc.tensor.matmul(
                            psums[n][h][:, s * sub:(s + 1) * sub]
                            if n_sub > 1 else psums[n][h],
                            w_tiles[n][:, k * P:(k + 1) * P],
                            xc[:, koff + h * MHS + s * sub:
                               koff + h * MHS + (s + 1) * sub],
                            start=(k == 0),
                            stop=(k == KT - 1),
                        )

            def evict(n):
                if not do_evict:
                    del psums[n]
                    return
                o_n = opool.tile([P, M_SHARD], dt_out, name=f"o{n}", tag="o")
                off = 0
                for i, (width, src) in enumerate(evsrc[n]):
                    if ev_half and i > 0:
                        break
                    dst = o_n[:, off:off + width]
                    if ev_eng == "act":
                        nc.scalar.copy(dst, src)
                    else:
                        nc.vector.tensor_copy(dst, src)
                    off += width
                if do_store:
                    eng = nc.sync if ev_eng == "act" else nc.scalar
                    eng.dma_start(out=out_r[:, n, :], in_=o_n[:])
                del psums[n], evsrc[n]

            # interleaved issue order: w0, x0, w1, x1, w2, x2, w3, x3,
            # x4..x7, then w4.. as the n-loop proceeds (wpool prefetch).
            GA = min(int(os.environ.get("KGA", "4")), NT)
            for i in range(max(GA, XCHUNKS)):
                if i < GA:
                    load_w(i)
                if i < XCHUNKS:
                    load_x(i)

            # Phase A: first GA n-tiles advance k-progressively per x chunk
            for n in range(GA):
                get_psums(n)
            for c in range(XCHUNKS):
                for n in range(GA):
                    for k in range(c * KPC, (c + 1) * KPC):
                        mm(n, k)
            for n in range(GA):
                evict(n)

            # Phase B: remaining n-tiles, serial (2 PSUM banks each).
            # The last n-tile runs m-half-outer so the first half's
            # eviction + store hide under the second half's matmuls,
            # shrinking the end-of-iteration tail.
            last_special = do_evict and n_sub == 1
            for n in range(GA, NT - 1 if last_special else NT):
                load_w(n)
                get_psums(n)
                for k in range(KT):
                    mm(n, k)
                evict(n)

            if last_special:
                n = NT - 1
                load_w(n)
                get_psums(n)
                o_n = opool.tile([P, M_SHARD], dt_out, name=f"o{n}",
                                 tag="o")
                for h in range(MH):
                    for k in range(KT):
                        xc = x_chunks[k // KPC]
                        koff = (k % KPC) * M_SHARD
                        nc.tensor.matmul(
                            psums[n][h],
                            w_tiles[n][:, k * P:(k + 1) * P],
                            xc[:, koff + h * MHS:koff + (h + 1) * MHS],
                            start=(k == 0),
                            stop=(k == KT - 1),
                        )
                    dst = o_n[:, h * MHS:(h + 1) * MHS]
                    if ev_eng == "act":
                        nc.scalar.copy(dst, psums[n][h])
                    else:
                        nc.vector.tensor_copy(dst, psums[n][h])
                    if do_store:
                        eng = nc.sync if ev_eng == "act" else nc.scalar
                        eng.dma_start(
                            out=out_r[:, n, h * MHS:(h + 1) * MHS],
                            in_=dst)
                del psums[n], evsrc[n]

        if timing:
            with tc.tile_pool(name="mpool", bufs=1) as mpool:
                m_t = mpool.tile([1, 64], mybir.dt.int32, name="mk", tag="mk")
                nc.gpsimd.memset(m_t[:], 0)
                nc.sync.dma_start(out=marker[:, :], in_=m_t[:])

    nc.compile()
    return nc


def _get_nc():
    global _cached_nc
    if _cached_nc is None:
        _cached_nc = _build()
    return _cached_nc


def _pack_x_shard(x_shard):
    """[M_SHARD, IN_DIM] -> [128, KT*M_SHARD] partition-major x^T tiles.

    dest[p, k*M_SHARD + m] = x_shard[m, k*128 + p]
    """
    return np.ascontiguousarray(
        x_shard.T.reshape(KT, P, M_SHARD).transpose(1, 0, 2).reshape(P, -1)
    )


def _pack_w(w_full):
    """[IN_DIM, OUT_DIM] -> [128, NT*KT*128] n-tile-major k-tile blocks.

    dest[p, (n*KT + k)*128 + j] = w_full[k*128 + p, n*128 + j]
    """
    return np.ascontiguousarray(
        w_full.reshape(KT, P, NT, P).transpose(1, 2, 0, 3).reshape(P, -1)
    )


def _make_in_maps(x, rules_outcome):
    np_dt = _np_dt()
    x = np.asarray(x, dtype=np_dt)
    w = np.asarray(rules_outcome, dtype=np_dt)
    assert x.shape == (BATCH, IN_DIM) and w.shape == (IN_DIM, OUT_DIM)
    w_packed = _pack_w(w)
    return [
        {
            "xt": _pack_x_shard(x[i * M_SHARD:(i + 1) * M_SHARD, :]),
            "w": w_packed,
        }
        for i in range(N_CORES)
    ]


def _run(x, rules_outcome, **spmd_kwargs):
    from concourse.bass_utils import run_bass_kernel_spmd

    in_maps = _make_in_maps(x, rules_outcome)
    nc = _get_nc()
    res = run_bass_kernel_spmd(nc, in_maps, core_ids=list(range(N_CORES)),
                               **spmd_kwargs)
    full = np.concatenate(
        [res.results[i]["out"].T for i in range(N_CORES)], axis=0)
    return np.ascontiguousarray(full, dtype=np.float32), res


def kernel(x, rules_outcome):
    out, _ = _run(x, rules_outcome)
    return out
